# revision 11
# baseline (speedup 1.0000x reference)
"""Trainium2 Bass kernel for GQA attention with RoPE (causal), tensor-parallel
over heads across 8 NeuronCores.

Reference computation (all fp32):
  q = (x @ wq.T)  -> [B,S,16,128], k/v = (x @ wk/wv.T) -> [B,S,4,128]
  q,k roped with interleaved-pair rotation; repeat_kv(4); causal softmax(qk/sqrt(128)) @ v
  out = attn @ wo.T

Sharding: core i handles q heads {2i, 2i+1} and kv head i//2 (exactly the kv
head those q heads attend to). wq/wk/wv are column-sharded, wo row-sharded;
the all-reduce over the 8 partial wo outputs happens on the host.

Host-side prep (layout only, no math): x is pre-transposed to [B,D,S];
the head_dim axis of wq/wk is permuted so RoPE pairs are de-interleaved
(real parts in rows 0..63, imag parts in rows 64..127 of each head) which
turns RoPE into ops on contiguous 64-partition slices. The score contraction
q.k is invariant to this permutation since q and k are permuted identically.
"""

import math
import os
import sys
from contextlib import ExitStack

import numpy as np

if "/opt/trn_rl_repo" not in sys.path:
    sys.path.insert(0, "/opt/trn_rl_repo")

B = 2
S = 2048
D = 2048
N_HEADS = 16
N_KV_HEADS = 4
HEAD_DIM = 128
N_CORES = 8
HPC = N_HEADS // N_CORES  # q heads per core = 2
SC = 512  # sequence chunk (matmul moving free dim)
NKO = D // 128  # contraction chunks for the projections = 16
NSB = S // 128  # 128-row seq blocks = 16
NCH = S // SC  # 512-wide seq chunks = 4
SCALE = 1.0 / math.sqrt(HEAD_DIM)

_CACHE = {}


def _build_module():
    import concourse.tile as tile
    from concourse import bacc, mybir
    from concourse.masks import make_identity

    f32 = mybir.dt.float32
    f32r = mybir.dt.float32r

    nc = bacc.Bacc(
        "TRN2",
        target_bir_lowering=False,
        debug=False,
        enable_asserts=False,
        num_devices=N_CORES,
    )
    xT = nc.dram_tensor("xT", [B, D, S], f32r, kind="ExternalInput").ap()
    wT = nc.dram_tensor("wT", [D, 512], f32r, kind="ExternalInput").ap()
    woT = nc.dram_tensor("woT", [2 * HEAD_DIM, D], f32r, kind="ExternalInput").ap()
    cs = nc.dram_tensor("cs", [128, S], f32, kind="ExternalInput").ap()
    mask = nc.dram_tensor("mask", [128, 1024], f32, kind="ExternalInput").ap()
    onesd = nc.dram_tensor("onesd", [128, 1], f32r, kind="ExternalInput").ap()
    y = nc.dram_tensor("y", [B, S, D], f32, kind="ExternalOutput").ap()

    with tile.TileContext(nc) as tc, ExitStack() as ctx:
        consts = ctx.enter_context(tc.tile_pool(name="consts", bufs=1))
        xp = ctx.enter_context(tc.tile_pool(name="xp", bufs=6))
        qk_pool = ctx.enter_context(tc.tile_pool(name="qk", bufs=1))
        v_pool = ctx.enter_context(tc.tile_pool(name="v", bufs=1))
        oT_pool = ctx.enter_context(tc.tile_pool(name="oT", bufs=1))
        es_pool = ctx.enter_context(tc.tile_pool(name="es", bufs=4))
        rope_tmp = ctx.enter_context(tc.tile_pool(name="ropetmp", bufs=2))
        r_pool = ctx.enter_context(tc.tile_pool(name="r", bufs=2))
        rb_pool = ctx.enter_context(tc.tile_pool(name="rb", bufs=2))
        y_pool = ctx.enter_context(tc.tile_pool(name="y", bufs=4))

        # weights / constants, split into <=256KB DMAs to spread across queues
        w_sb = consts.tile([128, NKO, 512], f32r)
        wT3 = wT.rearrange("(ko p) e -> p ko e", p=128)
        for ko in range(NKO):
            nc.sync.dma_start(w_sb[:, ko, :], wT3[:, ko, :])
        woT_sb = consts.tile([128, 2, D], f32r)
        woT3 = woT.rearrange("(ko p) f -> p ko f", p=128)
        for ko in range(2):
            for half in range(2):
                nc.sync.dma_start(
                    woT_sb[:, ko, half * 1024 : (half + 1) * 1024],
                    woT3[:, ko, half * 1024 : (half + 1) * 1024],
                )
        cs_sb = consts.tile([128, S], f32)
        for half in range(2):
            sl = slice(half * 1024, (half + 1) * 1024)
            nc.sync.dma_start(cs_sb[:, sl], cs[:, sl])
        mask_sb = consts.tile([128, 1024], f32)
        nc.sync.dma_start(mask_sb[:], mask)
        ones_sb = consts.tile([128, 1], f32r)
        nc.sync.dma_start(ones_sb[:], onesd)
        ident_sb = consts.tile([128, 128], f32)
        make_identity(nc, ident_sb[:])

        for b in range(B):
            qkT = qk_pool.tile([128, 3, S], f32r)  # [e, {q0,q1,k}, s]
            vT_sb = v_pool.tile([128, S], f32, tag="vT")  # [e, s]
            v_sb = v_pool.tile([128, NSB, 128], f32r, tag="v")  # [s_in_blk, blk, e]

            # ---- projections: qT/kT/vT in [e, s] layout ----
            with tc.tile_pool(name="ps_proj", bufs=2, space="PSUM") as ps_proj:
                for j in range(NCH):
                    sj = slice(SC * j, SC * (j + 1))
                    ps_q0 = ps_proj.tile([128, 512], f32, tag="q0")
                    ps_q1 = ps_proj.tile([128, 512], f32, tag="q1")
                    ps_k = ps_proj.tile([128, 512], f32, tag="k")
                    ps_vT = ps_proj.tile([128, 512], f32, tag="vT")
                    qko = [ps_q0, ps_q1, ps_k, ps_vT]
                    for ko in range(NKO):
                        x_t = xp.tile([128, 512], f32r)
                        nc.sync.dma_start(x_t[:], xT[b, 128 * ko : 128 * (ko + 1), sj])
                        st, sp = ko == 0, ko == NKO - 1
                        for c in range(4):
                            nc.tensor.matmul(
                                qko[c][:],
                                w_sb[:, ko, 128 * c : 128 * (c + 1)],
                                x_t[:],
                                start=st,
                                stop=sp,
                            )
                    # RoPE on q0/q1/k (de-interleaved pairs: rows 0..63 real,
                    # 64..127 imag; cs rows 0..63 cos, 64..127 sin)
                    for c in range(3):
                        ps = qko[c]
                        t1 = rope_tmp.tile([128, 512], f32, tag="t1")
                        t2 = rope_tmp.tile([128, 512], f32, tag="t2")
                        cc = cs_sb[0:64, sj]
                        ss = cs_sb[64:128, sj]
                        pr = ps[0:64, :]
                        pi = ps[64:128, :]
                        nc.vector.tensor_mul(t1[0:64, :], pr, cc)
                        nc.vector.tensor_mul(t1[64:128, :], pr, ss)
                        nc.vector.tensor_mul(t2[0:64, :], pi, ss)
                        nc.vector.tensor_mul(t2[64:128, :], pi, cc)
                        nc.vector.tensor_sub(qkT[0:64, c, sj], t1[0:64, :], t2[0:64, :])
                        nc.vector.tensor_add(
                            qkT[64:128, c, sj], t1[64:128, :], t2[64:128, :]
                        )
                    # vT: psum -> sbuf (ScalarE to keep DVE free)
                    nc.scalar.copy(vT_sb[:, sj], ps_vT[:])

            # ---- transpose vT -> v natural [s, e] via PE ----
            with tc.tile_pool(name="ps_vtr", bufs=2, space="PSUM") as ps_vtrp:
                for m in range(NSB):
                    ps_vtr = ps_vtrp.tile([128, 128], f32, tag="vtr")
                    nc.tensor.transpose(
                        ps_vtr[:], vT_sb[:, 128 * m : 128 * (m + 1)], ident_sb[:]
                    )
                    nc.scalar.copy(v_sb[:, m, :], ps_vtr[:])

            # ---- attention (scores^T layout: [sk, sq]; softmax over sk via
            # ones-matmul rowsums; no max subtraction -- |scores| <~ 5) ----
            oT = oT_pool.tile([128, HPC, S], f32r)  # [e, head, s]
            with tc.tile_pool(name="ps_attn", bufs=2, space="PSUM") as ps_attn:
                for h in range(HPC):
                    for c in range(NCH):
                        scj = slice(SC * c, SC * (c + 1))
                        nblk = 4 * (c + 1)
                        ps_o = ps_attn.tile([128, 512], f32, tag="o")
                        ps_r = ps_attn.tile([1, 512], f32, tag="r")
                        q_sl = qkT[:, h, scj]
                        es_tiles = {}
                        # software-pipelined: emit scores(jk+1) before pv(jk)
                        # so PE covers the exp latency of block jk
                        for jk in range(nblk + 1):
                            if jk < nblk:
                                ps_s = ps_attn.tile([128, 512], f32, tag="s")
                                nc.tensor.matmul(
                                    ps_s[:],
                                    qkT[:, 2, 128 * jk : 128 * (jk + 1)],
                                    q_sl,
                                    start=True,
                                    stop=True,
                                )
                                es = es_pool.tile([128, 512], f32r)
                                nc.scalar.activation(
                                    es[:],
                                    ps_s[:],
                                    mybir.ActivationFunctionType.Exp,
                                    scale=SCALE,
                                )
                                diag = jk - 4 * c
                                if diag >= 0:
                                    off = 128 * diag
                                    nc.vector.tensor_mul(
                                        es[:], es[:], mask_sb[:, 512 - off : 1024 - off]
                                    )
                                es_tiles[jk] = es
                            if jk >= 1:
                                pj = jk - 1
                                es = es_tiles.pop(pj)
                                st, sp = pj == 0, pj == nblk - 1
                                nc.tensor.matmul(
                                    ps_o[:],
                                    v_sb[:, pj, :],
                                    es[:],
                                    start=st,
                                    stop=sp,
                                )
                                nc.tensor.matmul(
                                    ps_r[:],
                                    ones_sb[:],
                                    es[:],
                                    start=st,
                                    stop=sp,
                                )
                        # normalize: oT[:, h, chunk] = ps_o * (1/rowsum)
                        r1 = r_pool.tile([1, 512], f32)
                        nc.vector.tensor_copy(r1[:], ps_r[:])
                        rb = rb_pool.tile([128, 512], f32)
                        nc.gpsimd.partition_broadcast(rb[:], r1[:])
                        nc.vector.reciprocal(rb[:], rb[:])
                        nc.vector.tensor_mul(oT[:, h, scj], ps_o[:], rb[:])

            # ---- output projection: y[s,f] = sum_e oT[e,s] * woT[e,f] ----
            with tc.tile_pool(name="ps_y", bufs=4, space="PSUM") as ps_yp:
                for m in range(NSB):
                    for fc in range(NCH):
                        fj = slice(SC * fc, SC * (fc + 1))
                        ps_y = ps_yp.tile([128, 512], f32, tag="y")
                        for e in range(2):
                            nc.tensor.matmul(
                                ps_y[:],
                                oT[:, e, 128 * m : 128 * (m + 1)],
                                woT_sb[:, e, fj],
                                start=(e == 0),
                                stop=(e == 1),
                            )
                        y_sb = y_pool.tile([128, 512], f32)
                        nc.scalar.copy(y_sb[:], ps_y[:])
                        nc.sync.dma_start(y[b, 128 * m : 128 * (m + 1), fj], y_sb[:])

    nc.compile()
    return nc


def _get_module():
    if "nc" not in _CACHE:
        _CACHE["nc"] = _build_module()
    return _CACHE["nc"]


def _prep_inputs(x, freqs_cos, freqs_sin, wq, wk, wv, wo):
    """Host-side shard/layout prep. Returns per-core input maps."""
    perm = np.concatenate([np.arange(0, 128, 2), np.arange(1, 128, 2)])
    xT = np.ascontiguousarray(x.transpose(0, 2, 1))  # [B, D, S]
    cs = np.concatenate(
        [np.ascontiguousarray(freqs_cos.T), np.ascontiguousarray(freqs_sin.T)], axis=0
    )  # [128, S]
    # big causal mask: mask[p, g] = 1.0 iff p <= g - 512
    p_idx = np.arange(128)[:, None]
    g_idx = np.arange(1024)[None, :]
    mask = (p_idx <= g_idx - 512).astype(np.float32)

    in_maps = []
    for i in range(N_CORES):
        wq_i = wq[256 * i : 256 * (i + 1)]  # [256, D] heads 2i, 2i+1
        wq_i = np.concatenate([wq_i[128 * h + perm] for h in range(HPC)], axis=0)
        kv = i // 2
        wk_i = wk[128 * kv : 128 * (kv + 1)][perm]  # [128, D]
        wv_i = wv[128 * kv : 128 * (kv + 1)]  # [128, D] (not permuted)
        wT_i = np.ascontiguousarray(
            np.concatenate([wq_i, wk_i, wv_i], axis=0).T
        )  # [D, 512]
        woT_i = np.ascontiguousarray(wo[:, 256 * i : 256 * (i + 1)].T)  # [256, D]
        in_maps.append(
            {
                "xT": xT,
                "wT": wT_i,
                "woT": woT_i,
                "cs": cs,
                "mask": mask,
                "onesd": np.ones((128, 1), dtype=np.float32),
            }
        )
    return in_maps


def kernel(x, freqs_cos, freqs_sin, wq, wk, wv, wo):
    from concourse.bass_utils import run_bass_kernel_spmd

    nc = _get_module()
    in_maps = _prep_inputs(x, freqs_cos, freqs_sin, wq, wk, wv, wo)
    res = run_bass_kernel_spmd(nc, in_maps, list(range(N_CORES)))
    out = np.zeros((B, S, D), dtype=np.float32)
    for i in range(N_CORES):
        out += res.results[i]["y"]
    return out


if __name__ == "__main__":
    nc = _get_module()
    print(
        "instructions:",
        sum(len(blk.instructions) for blk in nc.m.functions[0].blocks),
    )


# revision 24
# speedup vs baseline: 1.7799x; 1.7799x over previous
"""Trainium2 Bass kernel for GQA attention with RoPE (causal), tensor-parallel
over heads across 8 NeuronCores.

Reference computation (all fp32):
  q = (x @ wq.T)  -> [B,S,16,128], k/v = (x @ wk/wv.T) -> [B,S,4,128]
  q,k roped with interleaved-pair rotation; repeat_kv(4); causal softmax(qk/sqrt(128)) @ v
  out = attn @ wo.T

Sharding: core i handles q heads {2i, 2i+1} and kv head i//2 (exactly the kv
head those q heads attend to). wq/wk/wv are column-sharded, wo row-sharded;
the all-reduce over the 8 partial wo outputs happens on the host.

Host-side prep (layout only, no math): x is pre-transposed to [B,D,S];
the head_dim axis of wq/wk is permuted so RoPE pairs are de-interleaved
(real parts in rows 0..63, imag parts in rows 64..127 of each head) which
turns RoPE into ops on contiguous 64-partition slices. The score contraction
q.k is invariant to this permutation since q and k are permuted identically.
"""

import math
import os
import sys
from contextlib import ExitStack

import numpy as np

if "/opt/trn_rl_repo" not in sys.path:
    sys.path.insert(0, "/opt/trn_rl_repo")

B = 2
S = 2048
D = 2048
N_HEADS = 16
N_KV_HEADS = 4
HEAD_DIM = 128
N_CORES = 8
HPC = N_HEADS // N_CORES  # q heads per core = 2
SC = 512  # sequence chunk (matmul moving free dim)
NKO = D // 128  # contraction chunks for the projections = 16
NSB = S // 128  # 128-row seq blocks = 16
NCH = S // SC  # 512-wide seq chunks = 4
SCALE = 1.0 / math.sqrt(HEAD_DIM)

_CACHE = {}


def _build_module():
    import concourse.tile as tile
    from concourse import bacc, mybir
    from concourse.masks import make_identity

    f32 = mybir.dt.float32
    f32r = mybir.dt.float32r

    nc = bacc.Bacc(
        "TRN2",
        target_bir_lowering=False,
        debug=False,
        enable_asserts=False,
        num_devices=N_CORES,
    )
    xT = nc.dram_tensor("xT", [B, D, S], f32r, kind="ExternalInput").ap()
    wT = nc.dram_tensor("wT", [D, 512], f32r, kind="ExternalInput").ap()
    woT = nc.dram_tensor("woT", [2 * HEAD_DIM, D], f32r, kind="ExternalInput").ap()
    cs = nc.dram_tensor("cs", [128, S], f32, kind="ExternalInput").ap()
    mask = nc.dram_tensor("mask", [128, 1024], f32, kind="ExternalInput").ap()
    onesd = nc.dram_tensor("onesd", [128, 1], f32r, kind="ExternalInput").ap()
    y = nc.dram_tensor("y", [B, S, D], f32, kind="ExternalOutput").ap()

    with tile.TileContext(nc) as tc, ExitStack() as ctx:
        consts = ctx.enter_context(tc.tile_pool(name="consts", bufs=1))
        xp = ctx.enter_context(tc.tile_pool(name="xp", bufs=6))
        qk_pool = ctx.enter_context(tc.tile_pool(name="qk", bufs=1))
        v_pool = ctx.enter_context(tc.tile_pool(name="v", bufs=1))
        oT_pool = ctx.enter_context(tc.tile_pool(name="oT", bufs=1))
        es_pool = ctx.enter_context(tc.tile_pool(name="es", bufs=5))
        rope_tmp = ctx.enter_context(tc.tile_pool(name="ropetmp", bufs=2))
        r_pool = ctx.enter_context(tc.tile_pool(name="r", bufs=2))
        rb_pool = ctx.enter_context(tc.tile_pool(name="rb", bufs=2))
        y_pool = ctx.enter_context(tc.tile_pool(name="y", bufs=8))

        # constants are emitted lazily (at first use) so startup DMAs don't
        # delay the first projection matmuls
        w_sb = consts.tile([128, NKO, 512], f32r)
        wT3 = wT.rearrange("(ko p) e -> p ko e", p=128)
        woT_sb = consts.tile([128, 2, D], f32r)
        woT3 = woT.rearrange("(ko p) f -> p ko f", p=128)
        cs_sb = consts.tile([128, S], f32)
        mask_sb = consts.tile([128, 1024], f32)
        ones_sb = consts.tile([128, 1], f32r)
        ident_sb = consts.tile([128, 128], f32)

        def load_w_chunk(ko):
            nc.sync.dma_start(w_sb[:, ko, :], wT3[:, ko, :])

        def load_late_consts():
            for half in range(2):
                sl = slice(half * 1024, (half + 1) * 1024)
                nc.sync.dma_start(cs_sb[:, sl], cs[:, sl])
            nc.sync.dma_start(mask_sb[:], mask)
            nc.sync.dma_start(ones_sb[:], onesd)
            make_identity(nc, ident_sb[:])

        def load_woT():
            for ko in range(2):
                for half in range(2):
                    nc.sync.dma_start(
                        woT_sb[:, ko, half * 1024 : (half + 1) * 1024],
                        woT3[:, ko, half * 1024 : (half + 1) * 1024],
                    )

        for b in range(B):
            qkT = qk_pool.tile([128, 3, S], f32r)  # [e, {q0,q1,k}, s]
            vT_sb = v_pool.tile([128, S], f32, tag="vT")  # [e, s]
            v_sb = v_pool.tile([128, NSB, 128], f32r, tag="v")  # [s_in_blk, blk, e]

            # ---- projections: qT/kT/vT in [e, s] layout ----
            with tc.tile_pool(name="ps_proj", bufs=2, space="PSUM") as ps_proj:
                for j in range(NCH):
                    sj = slice(SC * j, SC * (j + 1))
                    ps_q0 = ps_proj.tile([128, 512], f32, tag="q0")
                    ps_q1 = ps_proj.tile([128, 512], f32, tag="q1")
                    ps_k = ps_proj.tile([128, 512], f32, tag="k")
                    ps_vT = ps_proj.tile([128, 512], f32, tag="vT")
                    qko = [ps_q0, ps_q1, ps_k, ps_vT]
                    for ko in range(NKO):
                        x_t = xp.tile([128, 512], f32r)
                        nc.sync.dma_start(x_t[:], xT[b, 128 * ko : 128 * (ko + 1), sj])
                        if b == 0 and j == 0:
                            load_w_chunk(ko)
                        st, sp = ko == 0, ko == NKO - 1
                        for c in range(4):
                            nc.tensor.matmul(
                                qko[c][:],
                                w_sb[:, ko, 128 * c : 128 * (c + 1)],
                                x_t[:],
                                start=st,
                                stop=sp,
                            )
                    if b == 0 and j == 0:
                        load_late_consts()
                    # RoPE on q0/q1/k (de-interleaved pairs: rows 0..63 real,
                    # 64..127 imag; cs rows 0..63 cos, 64..127 sin)
                    for c in range(3):
                        ps = qko[c]
                        t1 = rope_tmp.tile([128, 512], f32, tag="t1")
                        t2 = rope_tmp.tile([128, 512], f32, tag="t2")
                        cc = cs_sb[0:64, sj]
                        ss = cs_sb[64:128, sj]
                        pr = ps[0:64, :]
                        pi = ps[64:128, :]
                        nc.vector.tensor_mul(t1[0:64, :], pr, cc)
                        nc.vector.tensor_mul(t1[64:128, :], pr, ss)
                        nc.vector.tensor_mul(t2[0:64, :], pi, ss)
                        nc.vector.tensor_mul(t2[64:128, :], pi, cc)
                        nc.vector.tensor_sub(qkT[0:64, c, sj], t1[0:64, :], t2[0:64, :])
                        nc.vector.tensor_add(
                            qkT[64:128, c, sj], t1[64:128, :], t2[64:128, :]
                        )
                    # vT: psum -> sbuf (ScalarE to keep DVE free)
                    nc.scalar.copy(vT_sb[:, sj], ps_vT[:])

            # ---- transpose vT -> v natural [s, e] via PE ----
            with tc.tile_pool(name="ps_vtr", bufs=2, space="PSUM") as ps_vtrp:
                for m in range(NSB):
                    ps_vtr = ps_vtrp.tile([128, 128], f32, tag="vtr")
                    nc.tensor.transpose(
                        ps_vtr[:], vT_sb[:, 128 * m : 128 * (m + 1)], ident_sb[:]
                    )
                    nc.scalar.copy(v_sb[:, m, :], ps_vtr[:])

            # ---- attention (scores^T layout: [sk, sq]; softmax over sk via
            # ones-matmul rowsums; no max subtraction -- |scores| <~ 5) ----
            if b == 0:
                load_woT()
            oT = oT_pool.tile([128, HPC, S], f32r)  # [e, head, s]
            with (
                tc.tile_pool(name="ps_attn", bufs=2, space="PSUM") as ps_attn,
                tc.tile_pool(name="ps_attn_s", bufs=3, space="PSUM") as ps_attn_s,
                tc.tile_pool(name="ps_attn_o", bufs=3, space="PSUM") as ps_attn_o,
            ):
                for h in range(HPC):
                    for c in range(NCH):
                        scj = slice(SC * c, SC * (c + 1))
                        nblk = 4 * (c + 1)
                        ps_o = ps_attn_o.tile([128, 512], f32, tag="o")
                        ps_r = ps_attn.tile([1, 512], f32, tag="r")
                        q_sl = qkT[:, h, scj]
                        es_tiles = {}
                        # software-pipelined 2 deep: emit scores(jk+2) before
                        # pv(jk) so PE covers the exp+mask latency of block jk
                        PD = 2
                        for jk in range(nblk + PD):
                            if jk < nblk:
                                ps_s = ps_attn_s.tile([128, 512], f32, tag="s")
                                nc.tensor.matmul(
                                    ps_s[:],
                                    qkT[:, 2, 128 * jk : 128 * (jk + 1)],
                                    q_sl,
                                    start=True,
                                    stop=True,
                                )
                                es = es_pool.tile([128, 512], f32r)
                                nc.scalar.activation(
                                    es[:],
                                    ps_s[:],
                                    mybir.ActivationFunctionType.Exp,
                                    scale=SCALE,
                                )
                                diag = jk - 4 * c
                                if diag >= 0:
                                    off = 128 * diag
                                    nc.vector.tensor_mul(
                                        es[:], es[:], mask_sb[:, 512 - off : 1024 - off]
                                    )
                                es_tiles[jk] = es
                            if jk >= PD:
                                pj = jk - PD
                                es = es_tiles.pop(pj)
                                st, sp = pj == 0, pj == nblk - 1
                                nc.tensor.matmul(
                                    ps_o[:],
                                    v_sb[:, pj, :],
                                    es[:],
                                    start=st,
                                    stop=sp,
                                )
                                nc.tensor.matmul(
                                    ps_r[:],
                                    ones_sb[:],
                                    es[:],
                                    start=st,
                                    stop=sp,
                                )
                        # normalize: oT[:, h, chunk] = ps_o * (1/rowsum)
                        r1 = r_pool.tile([1, 512], f32)
                        nc.vector.tensor_copy(r1[:], ps_r[:])
                        rb = rb_pool.tile([128, 512], f32)
                        nc.gpsimd.partition_broadcast(rb[:], r1[:])
                        nc.vector.reciprocal(rb[:], rb[:])
                        nc.vector.tensor_mul(oT[:, h, scj], ps_o[:], rb[:])

            # ---- output projection: y[s,f] = sum_e oT[e,s] * woT[e,f] ----
            with tc.tile_pool(name="ps_y", bufs=6, space="PSUM") as ps_yp:
                for m in range(NSB):
                    for fc in range(NCH):
                        fj = slice(SC * fc, SC * (fc + 1))
                        ps_y = ps_yp.tile([128, 512], f32, tag="y")
                        for e in range(2):
                            nc.tensor.matmul(
                                ps_y[:],
                                oT[:, e, 128 * m : 128 * (m + 1)],
                                woT_sb[:, e, fj],
                                start=(e == 0),
                                stop=(e == 1),
                            )
                        y_sb = y_pool.tile([128, 512], f32)
                        # alternate copy engine to balance ACT/DVE load
                        if (m * NCH + fc) % 2 == 0:
                            nc.scalar.copy(y_sb[:], ps_y[:])
                        else:
                            nc.vector.tensor_copy(y_sb[:], ps_y[:])
                        nc.sync.dma_start(y[b, 128 * m : 128 * (m + 1), fj], y_sb[:])

    nc.compile()
    return nc


def _get_module():
    if "nc" not in _CACHE:
        _CACHE["nc"] = _build_module()
    return _CACHE["nc"]


def _prep_inputs(x, freqs_cos, freqs_sin, wq, wk, wv, wo):
    """Host-side shard/layout prep. Returns per-core input maps."""
    perm = np.concatenate([np.arange(0, 128, 2), np.arange(1, 128, 2)])
    xT = np.ascontiguousarray(x.transpose(0, 2, 1))  # [B, D, S]
    cs = np.concatenate(
        [np.ascontiguousarray(freqs_cos.T), np.ascontiguousarray(freqs_sin.T)], axis=0
    )  # [128, S]
    # big causal mask: mask[p, g] = 1.0 iff p <= g - 512
    p_idx = np.arange(128)[:, None]
    g_idx = np.arange(1024)[None, :]
    mask = (p_idx <= g_idx - 512).astype(np.float32)

    in_maps = []
    for i in range(N_CORES):
        wq_i = wq[256 * i : 256 * (i + 1)]  # [256, D] heads 2i, 2i+1
        wq_i = np.concatenate([wq_i[128 * h + perm] for h in range(HPC)], axis=0)
        kv = i // 2
        wk_i = wk[128 * kv : 128 * (kv + 1)][perm]  # [128, D]
        wv_i = wv[128 * kv : 128 * (kv + 1)]  # [128, D] (not permuted)
        wT_i = np.ascontiguousarray(
            np.concatenate([wq_i, wk_i, wv_i], axis=0).T
        )  # [D, 512]
        woT_i = np.ascontiguousarray(wo[:, 256 * i : 256 * (i + 1)].T)  # [256, D]
        in_maps.append(
            {
                "xT": xT,
                "wT": wT_i,
                "woT": woT_i,
                "cs": cs,
                "mask": mask,
                "onesd": np.ones((128, 1), dtype=np.float32),
            }
        )
    return in_maps


def kernel(x, freqs_cos, freqs_sin, wq, wk, wv, wo):
    from concourse.bass_utils import run_bass_kernel_spmd

    nc = _get_module()
    in_maps = _prep_inputs(x, freqs_cos, freqs_sin, wq, wk, wv, wo)
    res = run_bass_kernel_spmd(nc, in_maps, list(range(N_CORES)))
    out = np.zeros((B, S, D), dtype=np.float32)
    for i in range(N_CORES):
        out += res.results[i]["y"]
    return out


if __name__ == "__main__":
    nc = _get_module()
    print(
        "instructions:",
        sum(len(blk.instructions) for blk in nc.m.functions[0].blocks),
    )


# revision 26
# speedup vs baseline: 2.5383x; 1.4261x over previous
"""Trainium2 Bass kernel for GQA attention with RoPE (causal), tensor-parallel
over heads across 8 NeuronCores.

Reference computation (all fp32):
  q = (x @ wq.T)  -> [B,S,16,128], k/v = (x @ wk/wv.T) -> [B,S,4,128]
  q,k roped with interleaved-pair rotation; repeat_kv(4); causal softmax(qk/sqrt(128)) @ v
  out = attn @ wo.T

Sharding: core i handles q heads {2i, 2i+1} and kv head i//2 (exactly the kv
head those q heads attend to). wq/wk/wv are column-sharded, wo row-sharded;
the all-reduce over the 8 partial wo outputs happens on the host.

Host-side prep (layout only, no math): x is pre-transposed to [B,D,S];
the head_dim axis of wq/wk is permuted so RoPE pairs are de-interleaved
(real parts in rows 0..63, imag parts in rows 64..127 of each head) which
turns RoPE into ops on contiguous 64-partition slices. The score contraction
q.k is invariant to this permutation since q and k are permuted identically.
"""

import math
import os
import sys
from contextlib import ExitStack

import numpy as np

if "/opt/trn_rl_repo" not in sys.path:
    sys.path.insert(0, "/opt/trn_rl_repo")

B = 2
S = 2048
D = 2048
N_HEADS = 16
N_KV_HEADS = 4
HEAD_DIM = 128
N_CORES = 8
HPC = N_HEADS // N_CORES  # q heads per core = 2
SC = 512  # sequence chunk (matmul moving free dim)
NKO = D // 128  # contraction chunks for the projections = 16
NSB = S // 128  # 128-row seq blocks = 16
NCH = S // SC  # 512-wide seq chunks = 4
SCALE = 1.0 / math.sqrt(HEAD_DIM)

_CACHE = {}


def _build_module():
    import concourse.tile as tile
    from concourse import bacc, mybir
    from concourse.masks import make_identity

    f32 = mybir.dt.float32
    f32r = mybir.dt.float32r

    nc = bacc.Bacc(
        "TRN2",
        target_bir_lowering=False,
        debug=False,
        enable_asserts=False,
        num_devices=N_CORES,
    )
    xT = nc.dram_tensor("xT", [B, D, S], f32r, kind="ExternalInput").ap()
    wT = nc.dram_tensor("wT", [D, 512], f32r, kind="ExternalInput").ap()
    woT = nc.dram_tensor("woT", [2 * HEAD_DIM, D], f32r, kind="ExternalInput").ap()
    cs = nc.dram_tensor("cs", [128, S], f32, kind="ExternalInput").ap()
    mask = nc.dram_tensor("mask", [128, 1024], f32, kind="ExternalInput").ap()
    onesd = nc.dram_tensor("onesd", [128, 1], f32r, kind="ExternalInput").ap()
    y = nc.dram_tensor("y", [B, S, D], f32, kind="ExternalOutput").ap()

    with tile.TileContext(nc) as tc, ExitStack() as ctx:
        consts = ctx.enter_context(tc.tile_pool(name="consts", bufs=1))
        xp = ctx.enter_context(tc.tile_pool(name="xp", bufs=6))
        qk_pool = ctx.enter_context(tc.tile_pool(name="qk", bufs=1))
        v_pool = ctx.enter_context(tc.tile_pool(name="v", bufs=1))
        oT_pool = ctx.enter_context(tc.tile_pool(name="oT", bufs=1))
        es_pool = ctx.enter_context(tc.tile_pool(name="es", bufs=5))
        rope_tmp = ctx.enter_context(tc.tile_pool(name="ropetmp", bufs=2))
        r_pool = ctx.enter_context(tc.tile_pool(name="r", bufs=2))
        rb_pool = ctx.enter_context(tc.tile_pool(name="rb", bufs=2))
        y_pool = ctx.enter_context(tc.tile_pool(name="y", bufs=8))

        # constants are emitted lazily (at first use) so startup DMAs don't
        # delay the first projection matmuls
        w_sb = consts.tile([128, NKO, 512], f32r)
        wT3 = wT.rearrange("(ko p) e -> p ko e", p=128)
        woT_sb = consts.tile([128, 2, D], f32r)
        woT3 = woT.rearrange("(ko p) f -> p ko f", p=128)
        cs_sb = consts.tile([128, S], f32)
        mask_sb = consts.tile([128, 1024], f32)
        ones_sb = consts.tile([128, 1], f32r)
        ident_sb = consts.tile([128, 128], f32)

        def load_w_chunk(ko):
            nc.sync.dma_start(w_sb[:, ko, :], wT3[:, ko, :])

        def load_late_consts():
            for half in range(2):
                sl = slice(half * 1024, (half + 1) * 1024)
                nc.sync.dma_start(cs_sb[:, sl], cs[:, sl])
            nc.sync.dma_start(mask_sb[:], mask)
            nc.sync.dma_start(ones_sb[:], onesd)
            make_identity(nc, ident_sb[:])

        def load_woT():
            for ko in range(2):
                for half in range(2):
                    nc.sync.dma_start(
                        woT_sb[:, ko, half * 1024 : (half + 1) * 1024],
                        woT3[:, ko, half * 1024 : (half + 1) * 1024],
                    )

        for b in range(B):
            qkT = qk_pool.tile([128, 3, S], f32r)  # [e, {q0,q1,k}, s]
            vT_sb = v_pool.tile([128, S], f32, tag="vT")  # [e, s]
            v_sb = v_pool.tile([128, NSB, 128], f32r, tag="v")  # [s_in_blk, blk, e]

            # ---- projections: qT/kT/vT in [e, s] layout ----
            with tc.tile_pool(name="ps_proj", bufs=2, space="PSUM") as ps_proj:
                for j in range(NCH):
                    sj = slice(SC * j, SC * (j + 1))
                    ps_q0 = ps_proj.tile([128, 512], f32, tag="q0")
                    ps_q1 = ps_proj.tile([128, 512], f32, tag="q1")
                    ps_k = ps_proj.tile([128, 512], f32, tag="k")
                    ps_vT = ps_proj.tile([128, 512], f32, tag="vT")
                    qko = [ps_q0, ps_q1, ps_k, ps_vT]
                    for ko in range(NKO):
                        x_t = xp.tile([128, 512], f32r)
                        nc.sync.dma_start(x_t[:], xT[b, 128 * ko : 128 * (ko + 1), sj])
                        if b == 0 and j == 0:
                            load_w_chunk(ko)
                        st, sp = ko == 0, ko == NKO - 1
                        for c in range(4):
                            nc.tensor.matmul(
                                qko[c][:],
                                w_sb[:, ko, 128 * c : 128 * (c + 1)],
                                x_t[:],
                                start=st,
                                stop=sp,
                            )
                    if b == 0 and j == 0:
                        load_late_consts()
                    # RoPE on q0/q1/k (de-interleaved pairs: rows 0..63 real,
                    # 64..127 imag; cs rows 0..63 cos, 64..127 sin)
                    for c in range(3):
                        ps = qko[c]
                        t1 = rope_tmp.tile([128, 512], f32, tag="t1")
                        t2 = rope_tmp.tile([128, 512], f32, tag="t2")
                        cc = cs_sb[0:64, sj]
                        ss = cs_sb[64:128, sj]
                        pr = ps[0:64, :]
                        pi = ps[64:128, :]
                        nc.vector.tensor_mul(t1[0:64, :], pr, cc)
                        nc.vector.tensor_mul(t1[64:128, :], pr, ss)
                        nc.vector.tensor_mul(t2[0:64, :], pi, ss)
                        nc.vector.tensor_mul(t2[64:128, :], pi, cc)
                        nc.vector.tensor_sub(qkT[0:64, c, sj], t1[0:64, :], t2[0:64, :])
                        nc.vector.tensor_add(
                            qkT[64:128, c, sj], t1[64:128, :], t2[64:128, :]
                        )
                    # vT: psum -> sbuf (ScalarE to keep DVE free)
                    nc.scalar.copy(vT_sb[:, sj], ps_vT[:])

            # ---- transpose vT -> v natural [s, e] via PE ----
            with tc.tile_pool(name="ps_vtr", bufs=2, space="PSUM") as ps_vtrp:
                for m in range(NSB):
                    ps_vtr = ps_vtrp.tile([128, 128], f32, tag="vtr")
                    nc.tensor.transpose(
                        ps_vtr[:], vT_sb[:, 128 * m : 128 * (m + 1)], ident_sb[:]
                    )
                    nc.scalar.copy(v_sb[:, m, :], ps_vtr[:])

            # ---- attention (scores^T layout: [sk, sq]; softmax over sk via
            # ones-matmul rowsums; no max subtraction -- |scores| <~ 5) ----
            if b == 0:
                load_woT()
            oT = oT_pool.tile([128, HPC, S], f32r)  # [e, head, s]
            with (
                tc.tile_pool(name="ps_attn", bufs=1, space="PSUM") as ps_attn,
                tc.tile_pool(name="ps_attn_s", bufs=3, space="PSUM") as ps_attn_s,
                tc.tile_pool(name="ps_attn_o", bufs=2, space="PSUM") as ps_attn_o,
                tc.tile_pool(name="ps_y", bufs=2, space="PSUM") as ps_yp,
            ):
                # c-outer / h-inner so each s-chunk's wo projection (and its
                # 1MB y store) can issue right after both heads finish that
                # chunk, overlapping the remaining attention compute
                for c in range(NCH):
                    for h in range(HPC):
                        scj = slice(SC * c, SC * (c + 1))
                        nblk = 4 * (c + 1)
                        ps_o = ps_attn_o.tile([128, 512], f32, tag="o")
                        ps_r = ps_attn.tile([1, 512], f32, tag="r")
                        q_sl = qkT[:, h, scj]
                        es_tiles = {}
                        # software-pipelined 2 deep: emit scores(jk+2) before
                        # pv(jk) so PE covers the exp+mask latency of block jk
                        PD = 2
                        for jk in range(nblk + PD):
                            if jk < nblk:
                                ps_s = ps_attn_s.tile([128, 512], f32, tag="s")
                                nc.tensor.matmul(
                                    ps_s[:],
                                    qkT[:, 2, 128 * jk : 128 * (jk + 1)],
                                    q_sl,
                                    start=True,
                                    stop=True,
                                )
                                es = es_pool.tile([128, 512], f32r)
                                nc.scalar.activation(
                                    es[:],
                                    ps_s[:],
                                    mybir.ActivationFunctionType.Exp,
                                    scale=SCALE,
                                )
                                diag = jk - 4 * c
                                if diag >= 0:
                                    off = 128 * diag
                                    nc.vector.tensor_mul(
                                        es[:], es[:], mask_sb[:, 512 - off : 1024 - off]
                                    )
                                es_tiles[jk] = es
                            if jk >= PD:
                                pj = jk - PD
                                es = es_tiles.pop(pj)
                                st, sp = pj == 0, pj == nblk - 1
                                nc.tensor.matmul(
                                    ps_o[:],
                                    v_sb[:, pj, :],
                                    es[:],
                                    start=st,
                                    stop=sp,
                                )
                                nc.tensor.matmul(
                                    ps_r[:],
                                    ones_sb[:],
                                    es[:],
                                    start=st,
                                    stop=sp,
                                )
                        # normalize: oT[:, h, chunk] = ps_o * (1/rowsum)
                        r1 = r_pool.tile([1, 512], f32)
                        nc.vector.tensor_copy(r1[:], ps_r[:])
                        rb = rb_pool.tile([128, 512], f32)
                        nc.gpsimd.partition_broadcast(rb[:], r1[:])
                        nc.vector.reciprocal(rb[:], rb[:])
                        nc.vector.tensor_mul(oT[:, h, scj], ps_o[:], rb[:])

                    # ---- output projection for this s-chunk:
                    # y[s,f] = sum_e oT[e,s] * woT[e,f] ----
                    for mm in range(4):
                        m = 4 * c + mm
                        for fc in range(NCH):
                            fj = slice(SC * fc, SC * (fc + 1))
                            ps_y = ps_yp.tile([128, 512], f32, tag="y")
                            for e in range(2):
                                nc.tensor.matmul(
                                    ps_y[:],
                                    oT[:, e, 128 * m : 128 * (m + 1)],
                                    woT_sb[:, e, fj],
                                    start=(e == 0),
                                    stop=(e == 1),
                                )
                            y_sb = y_pool.tile([128, 512], f32)
                            # alternate copy engine to balance ACT/DVE load
                            if (m * NCH + fc) % 2 == 0:
                                nc.scalar.copy(y_sb[:], ps_y[:])
                            else:
                                nc.vector.tensor_copy(y_sb[:], ps_y[:])
                            nc.sync.dma_start(
                                y[b, 128 * m : 128 * (m + 1), fj], y_sb[:]
                            )

    nc.compile()
    return nc


def _get_module():
    if "nc" not in _CACHE:
        _CACHE["nc"] = _build_module()
    return _CACHE["nc"]


def _prep_inputs(x, freqs_cos, freqs_sin, wq, wk, wv, wo):
    """Host-side shard/layout prep. Returns per-core input maps."""
    perm = np.concatenate([np.arange(0, 128, 2), np.arange(1, 128, 2)])
    xT = np.ascontiguousarray(x.transpose(0, 2, 1))  # [B, D, S]
    cs = np.concatenate(
        [np.ascontiguousarray(freqs_cos.T), np.ascontiguousarray(freqs_sin.T)], axis=0
    )  # [128, S]
    # big causal mask: mask[p, g] = 1.0 iff p <= g - 512
    p_idx = np.arange(128)[:, None]
    g_idx = np.arange(1024)[None, :]
    mask = (p_idx <= g_idx - 512).astype(np.float32)

    in_maps = []
    for i in range(N_CORES):
        wq_i = wq[256 * i : 256 * (i + 1)]  # [256, D] heads 2i, 2i+1
        wq_i = np.concatenate([wq_i[128 * h + perm] for h in range(HPC)], axis=0)
        kv = i // 2
        wk_i = wk[128 * kv : 128 * (kv + 1)][perm]  # [128, D]
        wv_i = wv[128 * kv : 128 * (kv + 1)]  # [128, D] (not permuted)
        wT_i = np.ascontiguousarray(
            np.concatenate([wq_i, wk_i, wv_i], axis=0).T
        )  # [D, 512]
        woT_i = np.ascontiguousarray(wo[:, 256 * i : 256 * (i + 1)].T)  # [256, D]
        in_maps.append(
            {
                "xT": xT,
                "wT": wT_i,
                "woT": woT_i,
                "cs": cs,
                "mask": mask,
                "onesd": np.ones((128, 1), dtype=np.float32),
            }
        )
    return in_maps


def kernel(x, freqs_cos, freqs_sin, wq, wk, wv, wo):
    from concourse.bass_utils import run_bass_kernel_spmd

    nc = _get_module()
    in_maps = _prep_inputs(x, freqs_cos, freqs_sin, wq, wk, wv, wo)
    res = run_bass_kernel_spmd(nc, in_maps, list(range(N_CORES)))
    out = np.zeros((B, S, D), dtype=np.float32)
    for i in range(N_CORES):
        out += res.results[i]["y"]
    return out


if __name__ == "__main__":
    nc = _get_module()
    print(
        "instructions:",
        sum(len(blk.instructions) for blk in nc.m.functions[0].blocks),
    )


# revision 41
# speedup vs baseline: 8.0992x; 3.1907x over previous
"""Trainium2 Bass kernel for GQA attention with RoPE (causal), tensor-parallel
over heads across 8 NeuronCores.

Reference computation (all fp32):
  q = (x @ wq.T)  -> [B,S,16,128], k/v = (x @ wk/wv.T) -> [B,S,4,128]
  q,k roped with interleaved-pair rotation; repeat_kv(4); causal softmax(qk/sqrt(128)) @ v
  out = attn @ wo.T

Sharding: core i handles q heads {2i, 2i+1} and kv head i//2 (exactly the kv
head those q heads attend to). wq/wk/wv are column-sharded, wo row-sharded;
the all-reduce over the 8 partial wo outputs happens on the host.

Host-side prep (layout only, no math): x is pre-transposed to [B,D,S];
the head_dim axis of wq/wk is permuted so RoPE pairs are de-interleaved
(real parts in rows 0..63, imag parts in rows 64..127 of each head) which
turns RoPE into ops on contiguous 64-partition slices. The score contraction
q.k is invariant to this permutation since q and k are permuted identically.
"""

import math
import sys
from contextlib import ExitStack

import numpy as np

if "/opt/trn_rl_repo" not in sys.path:
    sys.path.insert(0, "/opt/trn_rl_repo")

B = 2
S = 2048
D = 2048
N_HEADS = 16
N_KV_HEADS = 4
HEAD_DIM = 128
N_CORES = 8
HPC = N_HEADS // N_CORES  # q heads per core = 2
SC = 512  # sequence chunk (matmul moving free dim)
NKO = D // 128  # contraction chunks for the projections = 16
NSB = S // 128  # 128-row seq blocks = 16
NCH = S // SC  # 512-wide seq chunks = 4
SCALE = 1.0 / math.sqrt(HEAD_DIM)

_CACHE = {}


def _build_module():
    import concourse.tile as tile
    from concourse import bacc, mybir
    from concourse.masks import make_identity

    f32 = mybir.dt.float32
    f32r = mybir.dt.float32r

    nc = bacc.Bacc(
        "TRN2",
        target_bir_lowering=False,
        debug=False,
        enable_asserts=False,
        num_devices=N_CORES,
    )
    xT = nc.dram_tensor("xT", [B, D, S], f32r, kind="ExternalInput").ap()
    wT = nc.dram_tensor("wT", [D, 512], f32r, kind="ExternalInput").ap()
    woT = nc.dram_tensor("woT", [2 * HEAD_DIM, D], f32r, kind="ExternalInput").ap()
    cs = nc.dram_tensor("cs", [128, S], f32, kind="ExternalInput").ap()
    mask = nc.dram_tensor("mask", [128, 1024], f32, kind="ExternalInput").ap()
    onesd = nc.dram_tensor("onesd", [128, 1], f32r, kind="ExternalInput").ap()
    y = nc.dram_tensor("y", [B, S, D], f32, kind="ExternalOutput").ap()

    with tile.TileContext(nc) as tc, ExitStack() as ctx:
        consts = ctx.enter_context(tc.tile_pool(name="consts", bufs=1))
        xp = ctx.enter_context(tc.tile_pool(name="xp", bufs=6))
        qk_pool = ctx.enter_context(tc.tile_pool(name="qk", bufs=1))
        v_pool = ctx.enter_context(tc.tile_pool(name="v", bufs=1))
        oT_pool = ctx.enter_context(tc.tile_pool(name="oT", bufs=1))
        es_pool = ctx.enter_context(tc.tile_pool(name="es", bufs=5))
        rope_tmp = ctx.enter_context(tc.tile_pool(name="ropetmp", bufs=2))
        qkraw_pool = ctx.enter_context(tc.tile_pool(name="qkraw", bufs=1))
        r_pool = ctx.enter_context(tc.tile_pool(name="r", bufs=2))
        rb_pool = ctx.enter_context(tc.tile_pool(name="rb", bufs=2))
        y_pool = ctx.enter_context(tc.tile_pool(name="y", bufs=8))

        # constants are emitted lazily (at first use) so startup DMAs don't
        # delay the first projection matmuls
        w_sb = consts.tile([128, NKO, 512], f32r)
        wT3 = wT.rearrange("(ko p) e -> p ko e", p=128)
        woT_sb = consts.tile([128, 2, D], f32r)
        woT3 = woT.rearrange("(ko p) f -> p ko f", p=128)
        cs_sb = consts.tile([128, S], f32)
        cs_swap = consts.tile([128, S], f32)  # halves swapped: [sin; cos]
        mask_sb = consts.tile([128, 1024], f32)
        ones_sb = consts.tile([128, 1], f32r)
        ident_sb = consts.tile([128, 128], f32)

        def load_w_chunk(ko):
            nc.sync.dma_start(w_sb[:, ko, :], wT3[:, ko, :])

        def load_late_consts():
            for half in range(2):
                sl = slice(half * 1024, (half + 1) * 1024)
                nc.sync.dma_start(cs_sb[:, sl], cs[:, sl])
            nc.sync.dma_start(cs_swap[0:64, :], cs_sb[64:128, :])
            nc.sync.dma_start(cs_swap[64:128, :], cs_sb[0:64, :])
            nc.sync.dma_start(mask_sb[:], mask)
            nc.sync.dma_start(ones_sb[:], onesd)
            make_identity(nc, ident_sb[:])

        def load_woT():
            for ko in range(2):
                for half in range(2):
                    nc.sync.dma_start(
                        woT_sb[:, ko, half * 1024 : (half + 1) * 1024],
                        woT3[:, ko, half * 1024 : (half + 1) * 1024],
                    )

        for b in range(B):
            qkT = qk_pool.tile([128, 3, S], f32r)  # [e, {q0,q1,k}, s]
            vT_sb = v_pool.tile([128, S], f32, tag="vT")  # [e, s]
            v_sb = v_pool.tile([128, NSB, 128], f32r, tag="v")  # [s_in_blk, blk, e]

            # ---- projections: qT/kT/vT in [e, s] layout ----
            with tc.tile_pool(name="ps_proj", bufs=2, space="PSUM") as ps_proj:
                for j in range(NCH):
                    sj = slice(SC * j, SC * (j + 1))
                    ps_q0 = ps_proj.tile([128, 512], f32, tag="q0")
                    ps_q1 = ps_proj.tile([128, 512], f32, tag="q1")
                    ps_k = ps_proj.tile([128, 512], f32, tag="k")
                    ps_vT = ps_proj.tile([128, 512], f32, tag="vT")
                    qko = [ps_q0, ps_q1, ps_k, ps_vT]
                    for ko in range(NKO):
                        x_t = xp.tile([128, 512], f32r)
                        nc.sync.dma_start(x_t[:], xT[b, 128 * ko : 128 * (ko + 1), sj])
                        if b == 0 and j == 0:
                            load_w_chunk(ko)
                        st, sp = ko == 0, ko == NKO - 1
                        for c in range(4):
                            nc.tensor.matmul(
                                qko[c][:],
                                w_sb[:, ko, 128 * c : 128 * (c + 1)],
                                x_t[:],
                                start=st,
                                stop=sp,
                            )
                    if b == 0 and j == 0:
                        load_late_consts()
                    # RoPE on q0/q1/k (de-interleaved pairs: rows 0..63 real,
                    # 64..127 imag; cs_sb = [cos; sin], cs_swap = [sin; cos]).
                    # For the LAST chunk of the batch, drain PSUM first via
                    # fast ACT copies (qk_raw=[p_r;p_i], qk_swap=[p_i;p_r]) so
                    # the proj pool releases without waiting on the slower DVE
                    # RoPE -- removes the ~10us PE stall at the phase boundary.
                    # (Every SB+SB DVE operand pair below shares its base
                    # partition, as the walrus verifier requires.)
                    last = j == NCH - 1
                    if last:
                        qk_raw = qkraw_pool.tile([128, 3, 512], f32, tag="qkraw")
                        qk_swap = qkraw_pool.tile([128, 3, 512], f32, tag="qkswap")
                        for c in range(3):
                            nc.scalar.copy(qk_raw[:, c, :], qko[c][:])
                            nc.scalar.copy(qk_swap[0:64, c, :], qko[c][64:128, :])
                            nc.scalar.copy(qk_swap[64:128, c, :], qko[c][0:64, :])
                    for c in range(3):
                        t1 = rope_tmp.tile([128, 512], f32, tag="t1")
                        t2 = rope_tmp.tile([128, 512], f32, tag="t2")
                        if last:
                            # t1 = [p_r*cos ; p_r*sin], t2 = [p_i*sin ; p_i*cos]
                            nc.vector.tensor_mul(
                                t1[0:64, :], qk_raw[0:64, c, :], cs_sb[0:64, sj]
                            )
                            nc.vector.tensor_mul(
                                t1[64:128, :], qk_swap[64:128, c, :], cs_sb[64:128, sj]
                            )
                            nc.vector.tensor_mul(
                                t2[0:64, :], qk_swap[0:64, c, :], cs_swap[0:64, sj]
                            )
                            nc.vector.tensor_mul(
                                t2[64:128, :], qk_raw[64:128, c, :], cs_swap[64:128, sj]
                            )
                        else:
                            ps = qko[c]
                            pr = ps[0:64, :]
                            pi = ps[64:128, :]
                            nc.vector.tensor_mul(t1[0:64, :], pr, cs_sb[0:64, sj])
                            nc.vector.tensor_mul(t1[64:128, :], pr, cs_sb[64:128, sj])
                            nc.vector.tensor_mul(t2[0:64, :], pi, cs_sb[64:128, sj])
                            nc.vector.tensor_mul(t2[64:128, :], pi, cs_sb[0:64, sj])
                        nc.vector.tensor_sub(qkT[0:64, c, sj], t1[0:64, :], t2[0:64, :])
                        nc.vector.tensor_add(
                            qkT[64:128, c, sj], t1[64:128, :], t2[64:128, :]
                        )
                    # vT: psum -> sbuf (ScalarE to keep DVE free)
                    nc.scalar.copy(vT_sb[:, sj], ps_vT[:])

            # ---- transpose vT -> v natural [s, e] via PE ----
            with tc.tile_pool(name="ps_vtr", bufs=2, space="PSUM") as ps_vtrp:
                for m in range(NSB):
                    ps_vtr = ps_vtrp.tile([128, 128], f32, tag="vtr")
                    nc.tensor.transpose(
                        ps_vtr[:], vT_sb[:, 128 * m : 128 * (m + 1)], ident_sb[:]
                    )
                    nc.scalar.copy(v_sb[:, m, :], ps_vtr[:])

            # ---- attention (scores^T layout: [sk, sq]; softmax over sk via
            # ones-matmul rowsums; no max subtraction -- |scores| <~ 5) ----
            if b == 0:
                load_woT()
            oT = oT_pool.tile([128, HPC, S], f32r)  # [e, head, s]
            with (
                tc.tile_pool(name="ps_attn", bufs=1, space="PSUM") as ps_attn,
                tc.tile_pool(name="ps_attn_s", bufs=3, space="PSUM") as ps_attn_s,
                tc.tile_pool(name="ps_attn_o", bufs=2, space="PSUM") as ps_attn_o,
                tc.tile_pool(name="ps_y", bufs=2, space="PSUM") as ps_yp,
            ):
                # c-outer / h-inner so each s-chunk's wo projection (and its
                # 1MB y store) can issue right after both heads finish that
                # chunk, overlapping the remaining attention compute
                for c in range(NCH):
                    for h in range(HPC):
                        scj = slice(SC * c, SC * (c + 1))
                        nblk = 4 * (c + 1)
                        ps_o = ps_attn_o.tile([128, 512], f32, tag="o")
                        ps_r = ps_attn.tile([1, 512], f32, tag="r")
                        q_sl = qkT[:, h, scj]
                        es_tiles = {}
                        # software-pipelined 2 deep: emit scores(jk+2) before
                        # pv(jk) so PE covers the exp+mask latency of block jk
                        PD = 2
                        for jk in range(nblk + PD):
                            if jk < nblk:
                                ps_s = ps_attn_s.tile([128, 512], f32, tag="s")
                                nc.tensor.matmul(
                                    ps_s[:],
                                    qkT[:, 2, 128 * jk : 128 * (jk + 1)],
                                    q_sl,
                                    start=True,
                                    stop=True,
                                )
                                es = es_pool.tile([128, 512], f32r)
                                nc.scalar.activation(
                                    es[:],
                                    ps_s[:],
                                    mybir.ActivationFunctionType.Exp,
                                    scale=SCALE,
                                )
                                diag = jk - 4 * c
                                if diag >= 0:
                                    off = 128 * diag
                                    nc.vector.tensor_mul(
                                        es[:], es[:], mask_sb[:, 512 - off : 1024 - off]
                                    )
                                es_tiles[jk] = es
                            if jk >= PD:
                                pj = jk - PD
                                es = es_tiles.pop(pj)
                                st, sp = pj == 0, pj == nblk - 1
                                nc.tensor.matmul(
                                    ps_o[:],
                                    v_sb[:, pj, :],
                                    es[:],
                                    start=st,
                                    stop=sp,
                                )
                                nc.tensor.matmul(
                                    ps_r[:],
                                    ones_sb[:],
                                    es[:],
                                    start=st,
                                    stop=sp,
                                )
                        # normalize: oT[:, h, chunk] = ps_o * (1/rowsum)
                        r1 = r_pool.tile([1, 512], f32)
                        nc.vector.tensor_copy(r1[:], ps_r[:])
                        rb = rb_pool.tile([128, 512], f32)
                        nc.gpsimd.partition_broadcast(rb[:], r1[:])
                        nc.vector.reciprocal(rb[:], rb[:])
                        nc.vector.tensor_mul(oT[:, h, scj], ps_o[:], rb[:])

                    # ---- output projection for this s-chunk:
                    # y[s,f] = sum_e oT[e,s] * woT[e,f] ----
                    for mm in range(4):
                        m = 4 * c + mm
                        for fc in range(NCH):
                            fj = slice(SC * fc, SC * (fc + 1))
                            ps_y = ps_yp.tile([128, 512], f32, tag="y")
                            for e in range(2):
                                nc.tensor.matmul(
                                    ps_y[:],
                                    oT[:, e, 128 * m : 128 * (m + 1)],
                                    woT_sb[:, e, fj],
                                    start=(e == 0),
                                    stop=(e == 1),
                                )
                            y_sb = y_pool.tile([128, 512], f32)
                            # alternate copy engine to balance ACT/DVE load
                            if (m * NCH + fc) % 2 == 0:
                                nc.scalar.copy(y_sb[:], ps_y[:])
                            else:
                                nc.vector.tensor_copy(y_sb[:], ps_y[:])
                            nc.sync.dma_start(
                                y[b, 128 * m : 128 * (m + 1), fj], y_sb[:]
                            )

    nc.compile()
    return nc


def _get_module():
    if "nc" not in _CACHE:
        _CACHE["nc"] = _build_module()
    return _CACHE["nc"]


def _prep_inputs(x, freqs_cos, freqs_sin, wq, wk, wv, wo):
    """Host-side shard/layout prep. Returns per-core input maps."""
    perm = np.concatenate([np.arange(0, 128, 2), np.arange(1, 128, 2)])
    xT = np.ascontiguousarray(x.transpose(0, 2, 1))  # [B, D, S]
    cs = np.concatenate(
        [np.ascontiguousarray(freqs_cos.T), np.ascontiguousarray(freqs_sin.T)], axis=0
    )  # [128, S]
    # big causal mask: mask[p, g] = 1.0 iff p <= g - 512
    p_idx = np.arange(128)[:, None]
    g_idx = np.arange(1024)[None, :]
    mask = (p_idx <= g_idx - 512).astype(np.float32)

    in_maps = []
    for i in range(N_CORES):
        wq_i = wq[256 * i : 256 * (i + 1)]  # [256, D] heads 2i, 2i+1
        wq_i = np.concatenate([wq_i[128 * h + perm] for h in range(HPC)], axis=0)
        kv = i // 2
        wk_i = wk[128 * kv : 128 * (kv + 1)][perm]  # [128, D]
        wv_i = wv[128 * kv : 128 * (kv + 1)]  # [128, D] (not permuted)
        wT_i = np.ascontiguousarray(
            np.concatenate([wq_i, wk_i, wv_i], axis=0).T
        )  # [D, 512]
        woT_i = np.ascontiguousarray(wo[:, 256 * i : 256 * (i + 1)].T)  # [256, D]
        in_maps.append(
            {
                "xT": xT,
                "wT": wT_i,
                "woT": woT_i,
                "cs": cs,
                "mask": mask,
                "onesd": np.ones((128, 1), dtype=np.float32),
            }
        )
    return in_maps


def kernel(x, freqs_cos, freqs_sin, wq, wk, wv, wo):
    from concourse.bass_utils import run_bass_kernel_spmd

    nc = _get_module()
    in_maps = _prep_inputs(x, freqs_cos, freqs_sin, wq, wk, wv, wo)
    res = run_bass_kernel_spmd(nc, in_maps, list(range(N_CORES)))
    out = np.zeros((B, S, D), dtype=np.float32)
    for i in range(N_CORES):
        out += res.results[i]["y"]
    return out


if __name__ == "__main__":
    nc = _get_module()
    print(
        "instructions:",
        sum(len(blk.instructions) for blk in nc.m.functions[0].blocks),
    )


# revision 42
# speedup vs baseline: 9.8242x; 1.2130x over previous
"""Trainium2 Bass kernel for GQA attention with RoPE (causal), tensor-parallel
over heads across 8 NeuronCores.

Reference computation (all fp32):
  q = (x @ wq.T)  -> [B,S,16,128], k/v = (x @ wk/wv.T) -> [B,S,4,128]
  q,k roped with interleaved-pair rotation; repeat_kv(4); causal softmax(qk/sqrt(128)) @ v
  out = attn @ wo.T

Sharding: core i handles q heads {2i, 2i+1} and kv head i//2 (exactly the kv
head those q heads attend to). wq/wk/wv are column-sharded, wo row-sharded;
the all-reduce over the 8 partial wo outputs happens on the host.

Host-side prep (layout only, no math): x is pre-transposed to [B,D,S];
the head_dim axis of wq/wk is permuted so RoPE pairs are de-interleaved
(real parts in rows 0..63, imag parts in rows 64..127 of each head) which
turns RoPE into ops on contiguous 64-partition slices. The score contraction
q.k is invariant to this permutation since q and k are permuted identically.
"""

import math
import sys
from contextlib import ExitStack

import numpy as np

if "/opt/trn_rl_repo" not in sys.path:
    sys.path.insert(0, "/opt/trn_rl_repo")

B = 2
S = 2048
D = 2048
N_HEADS = 16
N_KV_HEADS = 4
HEAD_DIM = 128
N_CORES = 8
HPC = N_HEADS // N_CORES  # q heads per core = 2
SC = 512  # sequence chunk (matmul moving free dim)
NKO = D // 128  # contraction chunks for the projections = 16
NSB = S // 128  # 128-row seq blocks = 16
NCH = S // SC  # 512-wide seq chunks = 4
SCALE = 1.0 / math.sqrt(HEAD_DIM)

_CACHE = {}


def _build_module():
    import concourse.tile as tile
    from concourse import bacc, mybir
    from concourse.masks import make_identity

    f32 = mybir.dt.float32
    f32r = mybir.dt.float32r

    nc = bacc.Bacc(
        "TRN2",
        target_bir_lowering=False,
        debug=False,
        enable_asserts=False,
        num_devices=N_CORES,
    )
    xT = nc.dram_tensor("xT", [B, D, S], f32r, kind="ExternalInput").ap()
    wT = nc.dram_tensor("wT", [D, 512], f32r, kind="ExternalInput").ap()
    woT = nc.dram_tensor("woT", [2 * HEAD_DIM, D], f32r, kind="ExternalInput").ap()
    cs = nc.dram_tensor("cs", [128, S], f32, kind="ExternalInput").ap()
    mask = nc.dram_tensor("mask", [128, 1024], f32, kind="ExternalInput").ap()
    onesd = nc.dram_tensor("onesd", [128, 1], f32r, kind="ExternalInput").ap()
    y = nc.dram_tensor("y", [B, S, D], f32, kind="ExternalOutput").ap()

    with tile.TileContext(nc) as tc, ExitStack() as ctx:
        consts = ctx.enter_context(tc.tile_pool(name="consts", bufs=1))
        xp = ctx.enter_context(tc.tile_pool(name="xp", bufs=6))
        qk_pool = ctx.enter_context(tc.tile_pool(name="qk", bufs=1))
        v_pool = ctx.enter_context(tc.tile_pool(name="v", bufs=1))
        oT_pool = ctx.enter_context(tc.tile_pool(name="oT", bufs=1))
        es_pool = ctx.enter_context(tc.tile_pool(name="es", bufs=5))
        rope_tmp = ctx.enter_context(tc.tile_pool(name="ropetmp", bufs=2))
        qkraw_pool = ctx.enter_context(tc.tile_pool(name="qkraw", bufs=1))
        r_pool = ctx.enter_context(tc.tile_pool(name="r", bufs=2))
        rb_pool = ctx.enter_context(tc.tile_pool(name="rb", bufs=2))
        y_pool = ctx.enter_context(tc.tile_pool(name="y", bufs=8))

        # constants are emitted lazily (at first use) so startup DMAs don't
        # delay the first projection matmuls
        w_sb = consts.tile([128, NKO, 512], f32r)
        wT3 = wT.rearrange("(ko p) e -> p ko e", p=128)
        woT_sb = consts.tile([128, 2, D], f32r)
        woT3 = woT.rearrange("(ko p) f -> p ko f", p=128)
        cs_sb = consts.tile([128, S], f32)
        cs_swap = consts.tile([128, S], f32)  # halves swapped: [sin; cos]
        mask_sb = consts.tile([128, 1024], f32)
        ones_sb = consts.tile([128, 1], f32r)
        ident_sb = consts.tile([128, 128], f32)

        def load_w_chunk(ko):
            nc.sync.dma_start(w_sb[:, ko, :], wT3[:, ko, :])

        def load_late_consts():
            for half in range(2):
                sl = slice(half * 1024, (half + 1) * 1024)
                nc.sync.dma_start(cs_sb[:, sl], cs[:, sl])
            nc.sync.dma_start(cs_swap[0:64, :], cs_sb[64:128, :])
            nc.sync.dma_start(cs_swap[64:128, :], cs_sb[0:64, :])
            nc.sync.dma_start(mask_sb[:], mask)
            nc.sync.dma_start(ones_sb[:], onesd)
            make_identity(nc, ident_sb[:])

        def load_woT():
            for ko in range(2):
                for half in range(2):
                    nc.sync.dma_start(
                        woT_sb[:, ko, half * 1024 : (half + 1) * 1024],
                        woT3[:, ko, half * 1024 : (half + 1) * 1024],
                    )

        for b in range(B):
            qkT = qk_pool.tile([128, 3, S], f32r)  # [e, {q0,q1,k}, s]
            vT_sb = v_pool.tile([128, S], f32, tag="vT")  # [e, s]
            v_sb = v_pool.tile([128, NSB, 128], f32r, tag="v")  # [s_in_blk, blk, e]

            # ---- projections: qT/kT/vT in [e, s] layout ----
            with tc.tile_pool(name="ps_proj", bufs=2, space="PSUM") as ps_proj:
                for j in range(NCH):
                    sj = slice(SC * j, SC * (j + 1))
                    ps_q0 = ps_proj.tile([128, 512], f32, tag="q0")
                    ps_q1 = ps_proj.tile([128, 512], f32, tag="q1")
                    ps_k = ps_proj.tile([128, 512], f32, tag="k")
                    ps_vT = ps_proj.tile([128, 512], f32, tag="vT")
                    qko = [ps_q0, ps_q1, ps_k, ps_vT]
                    for ko in range(NKO):
                        x_t = xp.tile([128, 512], f32r)
                        nc.sync.dma_start(x_t[:], xT[b, 128 * ko : 128 * (ko + 1), sj])
                        if b == 0 and j == 0:
                            load_w_chunk(ko)
                        st, sp = ko == 0, ko == NKO - 1
                        for c in range(4):
                            nc.tensor.matmul(
                                qko[c][:],
                                w_sb[:, ko, 128 * c : 128 * (c + 1)],
                                x_t[:],
                                start=st,
                                stop=sp,
                            )
                    if b == 0 and j == 0:
                        load_late_consts()
                    # RoPE on q0/q1/k (de-interleaved pairs: rows 0..63 real,
                    # 64..127 imag; cs_sb = [cos; sin], cs_swap = [sin; cos]).
                    # For the LAST chunk of the batch, drain PSUM first via
                    # fast ACT copies (qk_raw=[p_r;p_i], qk_swap=[p_i;p_r]) so
                    # the proj pool releases without waiting on the slower DVE
                    # RoPE -- removes the ~10us PE stall at the phase boundary.
                    # (Every SB+SB DVE operand pair below shares its base
                    # partition, as the walrus verifier requires.)
                    last = j == NCH - 1
                    if last:
                        qk_raw = qkraw_pool.tile([128, 3, 512], f32, tag="qkraw")
                        qk_swap = qkraw_pool.tile([128, 3, 512], f32, tag="qkswap")
                        for c in range(3):
                            nc.scalar.copy(qk_raw[:, c, :], qko[c][:])
                            nc.scalar.copy(qk_swap[0:64, c, :], qko[c][64:128, :])
                            nc.scalar.copy(qk_swap[64:128, c, :], qko[c][0:64, :])
                    for c in range(3):
                        t1 = rope_tmp.tile([128, 512], f32, tag="t1")
                        t2 = rope_tmp.tile([128, 512], f32, tag="t2")
                        if last:
                            # t1 = [p_r*cos ; p_r*sin], t2 = [p_i*sin ; p_i*cos]
                            nc.vector.tensor_mul(
                                t1[0:64, :], qk_raw[0:64, c, :], cs_sb[0:64, sj]
                            )
                            nc.vector.tensor_mul(
                                t1[64:128, :], qk_swap[64:128, c, :], cs_sb[64:128, sj]
                            )
                            nc.vector.tensor_mul(
                                t2[0:64, :], qk_swap[0:64, c, :], cs_swap[0:64, sj]
                            )
                            nc.vector.tensor_mul(
                                t2[64:128, :], qk_raw[64:128, c, :], cs_swap[64:128, sj]
                            )
                        else:
                            ps = qko[c]
                            pr = ps[0:64, :]
                            pi = ps[64:128, :]
                            nc.vector.tensor_mul(t1[0:64, :], pr, cs_sb[0:64, sj])
                            nc.vector.tensor_mul(t1[64:128, :], pr, cs_sb[64:128, sj])
                            nc.vector.tensor_mul(t2[0:64, :], pi, cs_sb[64:128, sj])
                            nc.vector.tensor_mul(t2[64:128, :], pi, cs_sb[0:64, sj])
                        nc.vector.tensor_sub(qkT[0:64, c, sj], t1[0:64, :], t2[0:64, :])
                        nc.vector.tensor_add(
                            qkT[64:128, c, sj], t1[64:128, :], t2[64:128, :]
                        )
                    # vT: psum -> sbuf (ScalarE to keep DVE free)
                    nc.scalar.copy(vT_sb[:, sj], ps_vT[:])

            # ---- transpose vT -> v natural [s, e] via PE ----
            with tc.tile_pool(name="ps_vtr", bufs=2, space="PSUM") as ps_vtrp:
                for m in range(NSB):
                    ps_vtr = ps_vtrp.tile([128, 128], f32, tag="vtr")
                    nc.tensor.transpose(
                        ps_vtr[:], vT_sb[:, 128 * m : 128 * (m + 1)], ident_sb[:]
                    )
                    nc.scalar.copy(v_sb[:, m, :], ps_vtr[:])

            # ---- attention (scores^T layout: [sk, sq]; softmax over sk via
            # ones-matmul rowsums; no max subtraction -- |scores| <~ 5) ----
            if b == 0:
                load_woT()
            oT = oT_pool.tile([128, HPC, S], f32r)  # [e, head, s]
            with (
                tc.tile_pool(name="ps_attn", bufs=1, space="PSUM") as ps_attn,
                tc.tile_pool(name="ps_attn_s", bufs=3, space="PSUM") as ps_attn_s,
                tc.tile_pool(name="ps_attn_o", bufs=2, space="PSUM") as ps_attn_o,
                tc.tile_pool(name="ps_y", bufs=2, space="PSUM") as ps_yp,
            ):
                # c-outer / h-inner so each s-chunk's wo projection (and its
                # 1MB y store) can issue right after both heads finish that
                # chunk, overlapping the remaining attention compute
                for c in range(NCH):
                    for h in range(HPC):
                        scj = slice(SC * c, SC * (c + 1))
                        nblk = 4 * (c + 1)
                        ps_o = ps_attn_o.tile([128, 512], f32, tag="o")
                        ps_r = ps_attn.tile([1, 512], f32, tag="r")
                        q_sl = qkT[:, h, scj]
                        es_tiles = {}
                        # software-pipelined 2 deep: emit scores(jk+2) before
                        # pv(jk) so PE covers the exp+mask latency of block jk
                        PD = 2
                        # diagonal blocks with offset >= 256 have their first
                        # 256 score columns fully masked: compute only the
                        # upper half (N=256) for scores/exp/mask/PV/rowsum.
                        def _lo(jk):
                            return 256 if 128 * (jk - 4 * c) >= 256 else 0

                        for jk in range(nblk + PD):
                            if jk < nblk:
                                lo = _lo(jk)
                                ps_s = ps_attn_s.tile([128, 512], f32, tag="s")
                                nc.tensor.matmul(
                                    ps_s[:, lo:512],
                                    qkT[:, 2, 128 * jk : 128 * (jk + 1)],
                                    q_sl[:, lo:512],
                                    start=True,
                                    stop=True,
                                )
                                es = es_pool.tile([128, 512], f32r)
                                nc.scalar.activation(
                                    es[:, lo:512],
                                    ps_s[:, lo:512],
                                    mybir.ActivationFunctionType.Exp,
                                    scale=SCALE,
                                )
                                diag = jk - 4 * c
                                if diag >= 0:
                                    off = 128 * diag
                                    nc.vector.tensor_mul(
                                        es[:, lo:512],
                                        es[:, lo:512],
                                        mask_sb[:, 512 - off + lo : 1024 - off],
                                    )
                                es_tiles[jk] = es
                            if jk >= PD:
                                pj = jk - PD
                                es = es_tiles.pop(pj)
                                lo = _lo(pj)
                                st, sp = pj == 0, pj == nblk - 1
                                nc.tensor.matmul(
                                    ps_o[:, lo:512],
                                    v_sb[:, pj, :],
                                    es[:, lo:512],
                                    start=st,
                                    stop=sp,
                                )
                                nc.tensor.matmul(
                                    ps_r[:, lo:512],
                                    ones_sb[:],
                                    es[:, lo:512],
                                    start=st,
                                    stop=sp,
                                )
                        # normalize: oT[:, h, chunk] = ps_o * (1/rowsum)
                        r1 = r_pool.tile([1, 512], f32)
                        nc.vector.tensor_copy(r1[:], ps_r[:])
                        rb = rb_pool.tile([128, 512], f32)
                        nc.gpsimd.partition_broadcast(rb[:], r1[:])
                        nc.vector.reciprocal(rb[:], rb[:])
                        nc.vector.tensor_mul(oT[:, h, scj], ps_o[:], rb[:])

                    # ---- output projection for this s-chunk:
                    # y[s,f] = sum_e oT[e,s] * woT[e,f] ----
                    for mm in range(4):
                        m = 4 * c + mm
                        for fc in range(NCH):
                            fj = slice(SC * fc, SC * (fc + 1))
                            ps_y = ps_yp.tile([128, 512], f32, tag="y")
                            for e in range(2):
                                nc.tensor.matmul(
                                    ps_y[:],
                                    oT[:, e, 128 * m : 128 * (m + 1)],
                                    woT_sb[:, e, fj],
                                    start=(e == 0),
                                    stop=(e == 1),
                                )
                            y_sb = y_pool.tile([128, 512], f32)
                            # alternate copy engine to balance ACT/DVE load
                            if (m * NCH + fc) % 2 == 0:
                                nc.scalar.copy(y_sb[:], ps_y[:])
                            else:
                                nc.vector.tensor_copy(y_sb[:], ps_y[:])
                            nc.sync.dma_start(
                                y[b, 128 * m : 128 * (m + 1), fj], y_sb[:]
                            )

    nc.compile()
    return nc


def _get_module():
    if "nc" not in _CACHE:
        _CACHE["nc"] = _build_module()
    return _CACHE["nc"]


def _prep_inputs(x, freqs_cos, freqs_sin, wq, wk, wv, wo):
    """Host-side shard/layout prep. Returns per-core input maps."""
    perm = np.concatenate([np.arange(0, 128, 2), np.arange(1, 128, 2)])
    xT = np.ascontiguousarray(x.transpose(0, 2, 1))  # [B, D, S]
    cs = np.concatenate(
        [np.ascontiguousarray(freqs_cos.T), np.ascontiguousarray(freqs_sin.T)], axis=0
    )  # [128, S]
    # big causal mask: mask[p, g] = 1.0 iff p <= g - 512
    p_idx = np.arange(128)[:, None]
    g_idx = np.arange(1024)[None, :]
    mask = (p_idx <= g_idx - 512).astype(np.float32)

    in_maps = []
    for i in range(N_CORES):
        wq_i = wq[256 * i : 256 * (i + 1)]  # [256, D] heads 2i, 2i+1
        wq_i = np.concatenate([wq_i[128 * h + perm] for h in range(HPC)], axis=0)
        kv = i // 2
        wk_i = wk[128 * kv : 128 * (kv + 1)][perm]  # [128, D]
        wv_i = wv[128 * kv : 128 * (kv + 1)]  # [128, D] (not permuted)
        wT_i = np.ascontiguousarray(
            np.concatenate([wq_i, wk_i, wv_i], axis=0).T
        )  # [D, 512]
        woT_i = np.ascontiguousarray(wo[:, 256 * i : 256 * (i + 1)].T)  # [256, D]
        in_maps.append(
            {
                "xT": xT,
                "wT": wT_i,
                "woT": woT_i,
                "cs": cs,
                "mask": mask,
                "onesd": np.ones((128, 1), dtype=np.float32),
            }
        )
    return in_maps


def kernel(x, freqs_cos, freqs_sin, wq, wk, wv, wo):
    from concourse.bass_utils import run_bass_kernel_spmd

    nc = _get_module()
    in_maps = _prep_inputs(x, freqs_cos, freqs_sin, wq, wk, wv, wo)
    res = run_bass_kernel_spmd(nc, in_maps, list(range(N_CORES)))
    out = np.zeros((B, S, D), dtype=np.float32)
    for i in range(N_CORES):
        out += res.results[i]["y"]
    return out


if __name__ == "__main__":
    nc = _get_module()
    print(
        "instructions:",
        sum(len(blk.instructions) for blk in nc.m.functions[0].blocks),
    )
